# revision 30
# baseline (speedup 1.0000x reference)
"""MixtureSageLayer Trainium2 kernel: scatter-mean GNN aggregation + 8-expert
dense layer + residual, sharded over 8 NeuronCores by destination node.

kernel(x, edge_index, weights) -> [100000, 8, 64] float32

Per-core plan (SPMD, one program, per-core data):
  - dest nodes sharded: core c owns rows [c*12500, (c+1)*12500)
  - edges routed to (batch of 8 dest-windows, source-chunk of 25000); within
    each (batch, chunk) gather segment the 8 windows' edge runs are packed
    back-to-back (slot counts = max over cores per (window, chunk) so the
    program structure is shared); segment tails are idx=0 pads whose dv=-1
    zeroes their Q rows
  - bulk dma_gather (int16 idxs, SWDGE) fetches 128B messages from a bf16
    row-duplicated copy of x into SBUF; the four source chunks map to the
    four SWDGE queues, whose Q7 core pairs generate descriptors CONCURRENTLY
    (~8 ns/idx per queue pair is the kernel's critical path)
  - per 128-slot tile "view" (a window's slots within one tile): one batched
    DVE is_equal builds Q[e, i] = (d[e]==i) in bf16 (pad slots d=-1 -> 0 row).
    The dv table is PAIR-DUPLICATED in memory and read through a 4-dim AP
    [p, v, (0, 64), (1, 2)] with a dense materialized iota, which keeps the
    bf16 pair-fetch legal and earns the DVE 2x perf mode (is_equal with a
    stride-0-inner broadcast src runs at 1x). TensorE accumulates
    psA[64, 128] = sum_e msg[e,:] * Q[e,:] per window (scatter-SUM)
  - per window: DVE multiplies psA by the per-dest inverse degree (scatter-mean
    deferred to the aggregate) into a bf16 aggT, then two bf16 matmuls compute
    out[128, 512] = aggT.T @ Wa + xT_w.T @ (Wx + I)  (the +I folds the
    residual), PSUM -> bf16 SBUF -> DMA to the bf16 output shard
  - software pipeline: gathers run two batches ahead and Q builds one batch
    ahead of the window compute (msg pool depth 3, q pool depth 3; xt/inv are
    streamed per batch to afford the q depth), hiding the drain burst and the
    late SWDGE completion semaphore under the matmul phase
"""
import numpy as np
import ml_dtypes

import concourse.bass as bass
import concourse.bacc as bacc
import concourse.mybir as mybir
import concourse.tile as tile
from concourse.bass_utils import run_bass_kernel_spmd

N_NODES = 100000
N_EDGES = 1000000
D = 64
K = 8
N_CORES = 8
NPC = N_NODES // N_CORES          # 12500 dest nodes per core
P = 128
NW = (NPC + P - 1) // P           # 98 windows per core
NPC_PAD = NW * P                  # 12544
WB = 8                            # nominal windows per gather batch
BATCH_SIZES = [8] * 12 + [2]
assert sum(BATCH_SIZES) == NW
NBATCH = len(BATCH_SIZES)         # 13
_BSTART = [sum(BATCH_SIZES[:i]) for i in range(NBATCH)]
NCH = 4                           # source chunks (int16 idx limit)
CH = 25000                        # chunk size
MBUFS_MSG = 4                     # msg pool depth
MBUFS_Q = 3                       # q pool depth
SINGLE_PACKET = False
PAD_MODE = "zero"                 # "trim": -1 tails + per-core count register

f32 = mybir.dt.float32
bf16 = mybir.dt.bfloat16
i16 = mybir.dt.int16

MAX_WAITS = 1

_CACHE = {}


def _split_sync_waits(nc, max_waits=MAX_WAITS):
    """The walrus codegen in this toolchain accepts very few sync-wait
    commands per instruction; spread extras onto preceding NoOps."""
    for f in nc.m.functions:
        for b in f.blocks:
            new_insts = []
            for inst in b.instructions:
                si = inst.sync_info
                if si is not None and si.on_wait and len(si.on_wait) > max_waits:
                    waits = list(si.on_wait)
                    chunks = [waits[i:i + max_waits]
                              for i in range(0, len(waits), max_waits)]
                    for k, chunk in enumerate(chunks[:-1]):
                        new_insts.append(mybir.InstNoOp(
                            name=f"{inst.name}-sw{k}",
                            sync_info=mybir.SyncInfo(on_wait=chunk, on_update=[]),
                            bass_nofuse=True,
                            engine=inst.engine,
                        ))
                    si.on_wait = chunks[-1]
                new_insts.append(inst)
            b.instructions = new_insts


def _dma_gather_128(g, out_ap, in_ap, idxs_ap, num_idxs, num_idxs_reg,
                    queue_num):
    """dma_gather with 128B elements on a 256B row stride.

    Mirrors bass's dma_gather lowering for the non-transpose DRAM-source
    path, which has no 256B element requirement in the ucode (one 128B
    descriptor per index; stride_bytes_256 stays integral at 256B).
    num_idxs_reg must equal this core's true element count (the position of
    the trailing -1 run) so the decode-side ring reservation matches what
    the Q7 generator pushes after value-trimming.
    """
    g._assert_queue_num(queue_num)
    _in_ap = g.lower_ap_dma(in_ap, for_custom_bir_dma=True)
    _idxs_ap = g.lower_ap(idxs_ap)
    _out_ap = g.lower_ap(out_ap)
    return g.add_instruction(
        mybir.InstDMAGatherAnt(
            name=g.bass.get_next_instruction_name(),
            ins=[*_in_ap, _idxs_ap, g.lower_val_access(g.to_reg(num_idxs_reg))],
            outs=[_out_ap],
            transpose=False,
            num_idxs=num_idxs,
            elem_size=D,
            stride_bytes_256=1,
            gen_mode=0,
            single_packet=SINGLE_PACKET,
            queue_num=queue_num,
            sbuf_tokens_per_rank=0,
            sbuf_free_dim_per_rank=0,
            sbuf_free_dim_pad_per_rank=0,
            sbuf_byte_offset=0,
        )
    )


def _wrap_idxs(seg):
    """dma_gather index layout: stream pos k -> [k % 16, k // 16], x8 tiled."""
    a = seg.reshape(-1, 16).T.astype(np.int16)
    return np.tile(a, (8, 1))


def _batch_windows(b):
    return list(range(_BSTART[b], _BSTART[b] + BATCH_SIZES[b]))


def _build_plan(edge_index):
    """Host-side routing: shared static structure + per-core padded streams."""
    row = np.asarray(edge_index[0], dtype=np.int64)
    col = np.asarray(edge_index[1], dtype=np.int64)

    deg = np.bincount(row, minlength=N_NODES).astype(np.float32)
    inv_global = (1.0 / np.maximum(deg, 1.0)).astype(np.float32)

    core_of = row // NPC
    per_core = []
    cnts = np.zeros((N_CORES, NW, NCH), np.int64)
    for c in range(N_CORES):
        sel = core_of == c
        rl = (row[sel] - c * NPC).astype(np.int64)
        cl = col[sel]
        w = rl >> 7
        j = cl // CH
        np.add.at(cnts[c], (w, j), 1)
        per_core.append((rl, cl, w, j))

    # per-core exact slot offsets within each (b, j) segment; segment tile
    # count shared = max over cores (per-core tails are -1 and ucode-trimmed)
    r0c = np.zeros((N_CORES, NW, NCH), np.int64)
    n_seg = np.zeros((N_CORES, NBATCH, NCH), np.int64)
    seg_len = {}
    seg_tiles = {}
    seg_t0 = {}
    t_acc = 0
    for b in range(NBATCH):
        ws = _batch_windows(b)
        for j in range(NCH):
            for c in range(N_CORES):
                o = 0
                for w in ws:
                    r0c[c, w, j] = o
                    o += int(cnts[c, w, j])
                n_seg[c, b, j] = o
            S = max(1, int(-(-n_seg[:, b, j].max() // P)))
            seg_len[(b, j)] = S * P
            seg_tiles[(b, j)] = S
            seg_t0[(b, j)] = t_acc
            t_acc += S
    T_total = t_acc

    # union views: per (w, j), tiles touched by ANY core's run
    views = []          # (b, j, w, seg_tile)
    view_id = {}
    seg_v0 = {}
    seg_nv = {}
    for b in range(NBATCH):
        ws = _batch_windows(b)
        for j in range(NCH):
            v0 = len(views)
            seg_v0[(b, j)] = v0
            for w in ws:
                lt_lo, lt_hi = None, None
                for c in range(N_CORES):
                    cnt = int(cnts[c, w, j])
                    if cnt == 0:
                        continue
                    a = int(r0c[c, w, j])
                    lo = a >> 7
                    hi = (a + cnt - 1) >> 7
                    lt_lo = lo if lt_lo is None else min(lt_lo, lo)
                    lt_hi = hi if lt_hi is None else max(lt_hi, hi)
                if lt_lo is None:
                    continue
                for lt in range(lt_lo, lt_hi + 1):
                    view_id[(w, j, lt)] = len(views)
                    views.append((b, j, w, lt))
            seg_nv[(b, j)] = len(views) - v0
    V_total = len(views)

    # per-window matmul schedule: (j, seg_tile, seg_view)
    sched = {w: [] for w in range(NW)}
    for vi, (b, j, w, lt) in enumerate(views):
        sched[w].append((j, lt, vi - seg_v0[(b, j)]))

    # per-core streams
    cores = []
    for c in range(N_CORES):
        rl, cl, w, j = per_core[c]
        key = (w * NCH + j).astype(np.int64)
        order = np.argsort(key, kind="stable")
        key_s = key[order]
        rl_s = rl[order]
        cl_s = cl[order]
        w_s = w[order]
        j_s = j[order]
        grp_start = np.zeros(NW * NCH, np.int64)
        np.cumsum(np.bincount(key, minlength=NW * NCH), out=grp_start[0:])
        grp_start = np.concatenate([[0], grp_start[:-1]])
        rank = np.arange(len(key_s)) - grp_start[key_s]

        w2b = np.zeros(NW, np.int64)
        for bb in range(NBATCH):
            w2b[_BSTART[bb]:_BSTART[bb] + BATCH_SIZES[bb]] = bb
        b_s = w2b[w_s]
        t0_s = np.array([seg_t0[(int(bb), int(jj))]
                         for bb, jj in zip(b_s, j_s)], np.int64)
        slot_in_seg = r0c[c, w_s, j_s] + rank
        pos = t0_s * P + slot_in_seg

        stream_idx = np.zeros(T_total * P, np.int16)
        if PAD_MODE == "trim":
            # first use of each msg buffer (batches < MBUFS_MSG) keeps
            # zero-pads so every buffer byte is initialized by a gather
            for b in range(MBUFS_MSG, NBATCH):
                for jj in range(NCH):
                    base = seg_t0[(b, jj)] * P
                    stream_idx[base + int(n_seg[c, b, jj]):
                               base + seg_len[(b, jj)]] = -1
        stream_idx[pos] = (cl_s - j_s * CH).astype(np.int16)

        dv = np.full((P, V_total), -1.0, np.float32)
        lt_s = (slot_in_seg >> 7).astype(np.int64)
        vi_s = np.array([view_id[(int(ww), int(jj), int(lt))]
                         for ww, jj, lt in zip(w_s, j_s, lt_s)], np.int64)
        dv[slot_in_seg & 127, vi_s] = (rl_s & 127).astype(np.float32)
        dv = np.repeat(dv, 2, axis=1)                    # pair-duplicated

        idx_parts = []
        for b in range(NBATCH):
            for jj in range(NCH):
                base = seg_t0[(b, jj)] * P
                idx_parts.append(
                    _wrap_idxs(stream_idx[base: base + seg_len[(b, jj)]]))
        idx_wrapped = np.concatenate(idx_parts, axis=1)
        dv_bf = dv.astype(ml_dtypes.bfloat16)
        nseg_c = np.zeros((1, NBATCH * NCH), np.int32)
        for b in range(NBATCH):
            for jj in range(NCH):
                nseg_c[0, b * NCH + jj] = n_seg[c, b, jj]
        cores.append((idx_wrapped, dv_bf, nseg_c))

    shared = dict(seg_len=seg_len, seg_tiles=seg_tiles, seg_t0=seg_t0,
                  seg_v0=seg_v0, seg_nv=seg_nv, sched=sched,
                  T_total=T_total, V_total=V_total)
    return shared, cores, inv_global


def _build_program(shared, do_compute=True):
    nc = bacc.Bacc("TRN2", num_swdge_queues=4)

    seg_len = shared["seg_len"]
    seg_tiles = shared["seg_tiles"]
    seg_v0 = shared["seg_v0"]
    seg_nv = shared["seg_nv"]
    sched = shared["sched"]
    V_total = shared["V_total"]

    ST16 = sum(seg_len.values()) // 16
    Smax = max(seg_tiles.values())

    Vmax = max(seg_nv.values())

    xdup_d = nc.dram_tensor("xdup", [N_NODES, 2 * D], bf16, kind="ExternalInput")
    nseg_d = nc.dram_tensor("nseg", [1, NBATCH * NCH], mybir.dt.int32,
                            kind="ExternalInput")
    xt_d = nc.dram_tensor("xt", [D, NPC_PAD], bf16, kind="ExternalInput")
    idx_d = nc.dram_tensor("idx", [P, ST16], i16, kind="ExternalInput")
    dv_d = nc.dram_tensor("dv", [P, 2 * V_total], bf16, kind="ExternalInput")
    iota_d = nc.dram_tensor("iota", [P, Vmax * P], bf16, kind="ExternalInput")
    inv_d = nc.dram_tensor("invr", [D, NPC_PAD], bf16, kind="ExternalInput")
    wa_d = nc.dram_tensor("wa", [D, K * D], bf16, kind="ExternalInput")
    wxi_d = nc.dram_tensor("wxi", [D, K * D], bf16, kind="ExternalInput")
    out_d = nc.dram_tensor("out", [NPC_PAD, K * D], bf16, kind="ExternalOutput")

    # idx tensor column offset of each segment
    seg_idx_off = {}
    o = 0
    for b in range(NBATCH):
        for j in range(NCH):
            seg_idx_off[(b, j)] = o
            o += seg_len[(b, j)] // 16

    with tile.TileContext(nc) as tc:
        with (
            tc.tile_pool(name="const", bufs=1) as cpool,
            tc.tile_pool(name="msg", bufs=MBUFS_MSG) as mpool,
            tc.tile_pool(name="q", bufs=MBUFS_Q) as qpool,
            tc.tile_pool(name="agg", bufs=4) as apool,
            tc.tile_pool(name="outs", bufs=3) as opool,
            tc.tile_pool(name="xtb", bufs=2) as xtpool,
            tc.tile_pool(name="invb", bufs=2) as invpool,
            tc.tile_pool(name="psA", bufs=4, space="PSUM") as pApool,
            tc.tile_pool(name="psB", bufs=2, space="PSUM") as pBpool,
        ):
            if PAD_MODE == "trim":
                nseg_t = cpool.tile([1, NBATCH * NCH], mybir.dt.int32)
                nc.sync.dma_start(out=nseg_t[:], in_=nseg_d[:])
            # per-batch idx tiles so the first gather isn't gated on the
            # whole idx stream landing; batch 0 additionally splits per
            # chunk so gather(0, q0) starts after just its own slice
            idx_bt = []
            idx0 = {}
            bo = 0
            for b in range(NBATCH):
                blen = sum(seg_len[(b, j)] for j in range(NCH)) // 16
                if b == 0:
                    for j in range(NCH):
                        jo = seg_idx_off[(0, j)]
                        jl = seg_len[(0, j)] // 16
                        tj = cpool.tile([P, jl], i16, tag=f"idx0j{j}")
                        nc.sync.dma_start(out=tj[:], in_=idx_d[:, jo:jo + jl])
                        idx0[j] = tj
                    idx_bt.append((None, bo))
                else:
                    t = cpool.tile([P, blen], i16, tag=f"idx{b}")
                    nc.sync.dma_start(out=t[:], in_=idx_d[:, bo:bo + blen])
                    idx_bt.append((t, bo))
                bo += blen
            dv_t = cpool.tile([P, V_total, 2], bf16)
            iota_t = cpool.tile([P, Vmax, P], bf16)
            wa_t = cpool.tile([D, K * D], bf16)
            wxi_t = cpool.tile([D, K * D], bf16)
            _dv = dv_t[:]
            nc.sync.dma_start(
                out=bass.AP(_dv.tensor, _dv.offset,
                            [_dv.ap[0], [1, 2 * V_total]]),
                in_=dv_d[:])
            _it = iota_t[:]
            nc.sync.dma_start(
                out=bass.AP(_it.tensor, _it.offset,
                            [_it.ap[0], [1, Vmax * P]]),
                in_=iota_d[:])
            nc.sync.dma_start(out=wa_t[:], in_=wa_d[:])
            nc.sync.dma_start(out=wxi_t[:], in_=wxi_d[:])

            if PAD_MODE == "trim":
                # guard only the tail tiles beyond what the first-use batch
                # (which is fully zero-padded) writes; everything else is
                # always gather-initialized before it can be read
                for j in range(NCH):
                    for k in range(MBUFS_MSG):
                        t = mpool.tile([P, Smax, D], bf16, tag=f"msg{j}")
                        s0 = seg_tiles[(k, j)]
                        if s0 < Smax:
                            nc.vector.memset(t[:, s0:, :], 0.0)

            tiles_b = {}

            def emit_gathers(b):
                msg_bj = {}
                for j in range(NCH):
                    S = seg_tiles[(b, j)]
                    L = seg_len[(b, j)]
                    if b == 0:
                        idx_t, io = idx0[j], 0
                    else:
                        idx_t, bo = idx_bt[b]
                        io = seg_idx_off[(b, j)] - bo
                    msg_t = mpool.tile([P, Smax, D], bf16, tag=f"msg{j}")
                    msg_bj[j] = msg_t
                    if PAD_MODE == "trim" and b >= MBUFS_MSG:
                        nreg = nc.gpsimd.alloc_register(f"nseg_{b}_{j}")
                        nc.gpsimd.reg_load(
                            nreg, nseg_t[0:1, b * NCH + j:b * NCH + j + 1])
                    else:
                        nreg = L
                    _dma_gather_128(
                        nc.gpsimd,
                        msg_t[:, :S, :],
                        xdup_d[CH * j:, 0:D] if j else xdup_d[:, 0:D],
                        idx_t[:, io:io + L // 16],
                        L,
                        nreg,
                        queue_num=j,
                    )
                tiles_b[b] = [msg_bj, None]

            def emit_iseq(b):
                w0 = _BSTART[b]
                nwb = BATCH_SIZES[b]
                xtb = xtpool.tile([D, WB * P], bf16, tag="xtb")
                invb = invpool.tile([D, WB * P], bf16, tag="invb")
                nc.sync.dma_start(out=xtb[:, :nwb * P],
                                  in_=xt_d[:, w0 * P:(w0 + nwb) * P])
                nc.sync.dma_start(out=invb[:, :nwb * P],
                                  in_=inv_d[:, w0 * P:(w0 + nwb) * P])
                q_bj = {}
                for j in range(NCH):
                    nv = seg_nv[(b, j)]
                    v0 = seg_v0[(b, j)]
                    q_t = qpool.tile([P, Vmax, P], bf16, tag=f"q{j}")
                    q_bj[j] = q_t
                    if nv and do_compute:
                        # pair-duplicated dv + 4-dim AP [p, v, (0, 64), (1, 2)]
                        # keeps the bf16 pair-fetch legal -> DVE 2x mode
                        dcols = dv_t[:, v0:v0 + nv, :]
                        d_b = bass.AP(dcols.tensor, dcols.offset,
                                      [dcols.ap[0], dcols.ap[1],
                                       [0, P // 2], dcols.ap[2]])
                        nc.vector.tensor_tensor(out=q_t[:, :nv, :],
                                                in0=iota_t[:, :nv, :],
                                                in1=d_b,
                                                op=mybir.AluOpType.is_equal)
                tiles_b[b][1] = (q_bj, xtb, invb)

            def emit_windows(b):
                msg_bj, (q_bj, xtb, invb) = tiles_b.pop(b)
                w0 = _BSTART[b]
                for w in _batch_windows(b):
                    if not do_compute:
                        continue
                    entries = sched[w]
                    psA = pApool.tile([D, P], f32, space="PSUM", tag="psA")
                    aggT = apool.tile([D, P], bf16, tag="agg")
                    if entries:
                        for k, (j, lt, svi) in enumerate(entries):
                            nc.tensor.matmul(
                                out=psA[:],
                                lhsT=msg_bj[j][:, lt, :],
                                rhs=q_bj[j][:, svi, :],
                                start=(k == 0),
                                stop=(k == len(entries) - 1),
                            )
                        nc.vector.tensor_tensor(
                            out=aggT[:], in0=psA[:],
                            in1=invb[:, (w - w0) * P:(w - w0 + 1) * P],
                            op=mybir.AluOpType.mult)
                    else:
                        nc.vector.memset(aggT[:], 0.0)

                    psB = pBpool.tile([P, K * D], f32, space="PSUM", tag="psB")
                    nc.tensor.matmul(out=psB[:], lhsT=aggT[:], rhs=wa_t[:],
                                     start=True, stop=False)
                    nc.tensor.matmul(out=psB[:],
                                     lhsT=xtb[:, (w - w0) * P:(w - w0 + 1) * P],
                                     rhs=wxi_t[:], start=False, stop=True)
                    out_t = opool.tile([P, K * D], bf16, tag="out")
                    nc.scalar.mul(out_t[:], psB[:], 1.0)
                    nc.sync.dma_start(out=out_d[w * P:(w + 1) * P, :],
                                      in_=out_t[:])

            # software pipeline: gathers run TWO batches ahead of the window
            # compute (covering both the SWDGE drain burst and the late DMA
            # completion semaphore), Q builds one batch ahead
            emit_gathers(0)
            emit_gathers(1)
            emit_iseq(0)
            for b in range(NBATCH):
                if b + 2 < NBATCH - 1:
                    emit_gathers(b + 2)
                elif b + 2 == NBATCH - 1:
                    emit_gathers(NBATCH - 2)
                    emit_gathers(NBATCH - 1)
                if b + 1 < NBATCH:
                    emit_iseq(b + 1)
                emit_windows(b)

    nc.compile()
    _split_sync_waits(nc)
    return nc


def kernel(x, edge_index, weights):
    x = np.asarray(x, dtype=np.float32)
    weights = np.asarray(weights, dtype=np.float32)

    shared, cores, inv_global = _build_plan(edge_index)

    shape_key = (shared["T_total"], shared["V_total"],
                 tuple(sorted(shared["seg_len"].items())),
                 tuple((w, tuple(v)) for w, v in sorted(shared["sched"].items())))
    if shape_key in _CACHE:
        nc = _CACHE[shape_key]
    else:
        nc = _build_program(shared)
        _CACHE.clear()
        _CACHE[shape_key] = nc

    x_bf = x.astype(ml_dtypes.bfloat16)
    xdup = np.concatenate([x_bf, x_bf], axis=1)          # [N, 128] bf16

    vmax = max(shared["seg_nv"].values())
    iota = np.broadcast_to(
        np.arange(P, dtype=np.float32),
        (P, vmax, P)).reshape(P, vmax * P).astype(ml_dtypes.bfloat16)

    wa = np.ascontiguousarray(
        weights[:, :D, :].transpose(1, 0, 2).reshape(D, K * D)
    ).astype(ml_dtypes.bfloat16)
    wx = weights[:, D:, :].transpose(1, 0, 2).reshape(D, K * D).copy()
    eye = np.eye(D, dtype=np.float32)
    for k in range(K):
        wx[:, k * D:(k + 1) * D] += eye
    wxi = wx.astype(ml_dtypes.bfloat16)

    in_maps = []
    for c in range(N_CORES):
        idx_wrapped, dv_bf, nseg_c = cores[c]
        xt = np.zeros((D, NPC_PAD), np.float32)
        xt[:, :NPC] = x[c * NPC:(c + 1) * NPC].T
        invr = np.zeros((D, NPC_PAD), np.float32)
        invr[:, :NPC] = inv_global[None, c * NPC:(c + 1) * NPC]
        in_maps.append({
            "xdup": xdup,
            "nseg": nseg_c,
            "xt": xt.astype(ml_dtypes.bfloat16),
            "idx": idx_wrapped,
            "dv": dv_bf,
            "iota": np.ascontiguousarray(iota),
            "invr": invr.astype(ml_dtypes.bfloat16),
            "wa": wa,
            "wxi": wxi,
        })

    res = run_bass_kernel_spmd(nc, in_maps, core_ids=list(range(N_CORES)))

    out = np.empty((N_NODES, K, D), np.float32)
    for c in range(N_CORES):
        oc = res.results[c]["out"][:NPC].astype(np.float32)  # [12500, 512]
        out[c * NPC:(c + 1) * NPC] = oc.reshape(NPC, K, D)
    return out



# revision 31
# speedup vs baseline: 1.1119x; 1.1119x over previous
"""MixtureSageLayer Trainium2 kernel: scatter-mean GNN aggregation + 8-expert
dense layer + residual, sharded over 8 NeuronCores by destination node.

kernel(x, edge_index, weights) -> [100000, 8, 64] float32

Per-core plan (SPMD, one program, per-core data):
  - dest nodes sharded: core c owns rows [c*12500, (c+1)*12500)
  - edges routed to (batch of 8 dest-windows, source-chunk of 25000); within
    each (batch, chunk) gather segment the 8 windows' edge runs are packed
    back-to-back (slot counts = max over cores per (window, chunk) so the
    program structure is shared); segment tails are idx=0 pads whose dv=-1
    zeroes their Q rows
  - bulk dma_gather (int16 idxs, SWDGE) fetches 128B messages from a bf16
    row-duplicated copy of x into SBUF; the four source chunks map to the
    four SWDGE queues, whose Q7 core pairs generate descriptors CONCURRENTLY
    (~8 ns/idx per queue pair is the kernel's critical path)
  - per 128-slot tile "view" (a window's slots within one tile): one batched
    DVE is_equal builds Q[e, i] = (d[e]==i) in bf16 (pad slots d=-1 -> 0 row).
    The dv table is PAIR-DUPLICATED in memory and read through a 4-dim AP
    [p, v, (0, 64), (1, 2)] with a dense materialized iota, which keeps the
    bf16 pair-fetch legal and earns the DVE 2x perf mode (is_equal with a
    stride-0-inner broadcast src runs at 1x). TensorE accumulates
    psA[64, 128] = sum_e msg[e,:] * Q[e,:] per window (scatter-SUM)
  - per window: DVE multiplies psA by the per-dest inverse degree (scatter-mean
    deferred to the aggregate) into a bf16 aggT, then two bf16 matmuls compute
    out[128, 512] = aggT.T @ Wa + xT_w.T @ (Wx + I)  (the +I folds the
    residual), PSUM -> bf16 SBUF -> DMA to the bf16 output shard
  - software pipeline: gathers run two batches ahead and Q builds one batch
    ahead of the window compute (msg pool depth 3, q pool depth 3; xt/inv are
    streamed per batch to afford the q depth), hiding the drain burst and the
    late SWDGE completion semaphore under the matmul phase
"""
import numpy as np
import ml_dtypes

import concourse.bass as bass
import concourse.bacc as bacc
import concourse.mybir as mybir
import concourse.tile as tile
from concourse.bass_utils import run_bass_kernel_spmd

N_NODES = 100000
N_EDGES = 1000000
D = 64
K = 8
N_CORES = 8
NPC = N_NODES // N_CORES          # 12500 dest nodes per core
P = 128
NW = (NPC + P - 1) // P           # 98 windows per core
NPC_PAD = NW * P                  # 12544
WB = 8                            # nominal windows per gather batch
BATCH_SIZES = [8] * 12 + [2]
assert sum(BATCH_SIZES) == NW
NBATCH = len(BATCH_SIZES)         # 13
_BSTART = [sum(BATCH_SIZES[:i]) for i in range(NBATCH)]
NCH = 4                           # source chunks (int16 idx limit)
CH = 25000                        # chunk size
MBUFS_MSG = 3                     # msg pool depth
MBUFS_Q = 3                       # q pool depth
SINGLE_PACKET = False
PAD_MODE = "zero"                 # "trim": -1 tails + per-core count register

f32 = mybir.dt.float32
bf16 = mybir.dt.bfloat16
i16 = mybir.dt.int16

MAX_WAITS = 1

_CACHE = {}


def _split_sync_waits(nc, max_waits=MAX_WAITS):
    """The walrus codegen in this toolchain accepts very few sync-wait
    commands per instruction; spread extras onto preceding NoOps."""
    for f in nc.m.functions:
        for b in f.blocks:
            new_insts = []
            for inst in b.instructions:
                si = inst.sync_info
                if si is not None and si.on_wait and len(si.on_wait) > max_waits:
                    waits = list(si.on_wait)
                    chunks = [waits[i:i + max_waits]
                              for i in range(0, len(waits), max_waits)]
                    for k, chunk in enumerate(chunks[:-1]):
                        new_insts.append(mybir.InstNoOp(
                            name=f"{inst.name}-sw{k}",
                            sync_info=mybir.SyncInfo(on_wait=chunk, on_update=[]),
                            bass_nofuse=True,
                            engine=inst.engine,
                        ))
                    si.on_wait = chunks[-1]
                new_insts.append(inst)
            b.instructions = new_insts


def _dma_gather_128(g, out_ap, in_ap, idxs_ap, num_idxs, num_idxs_reg,
                    queue_num):
    """dma_gather with 128B elements on a 256B row stride.

    Mirrors bass's dma_gather lowering for the non-transpose DRAM-source
    path, which has no 256B element requirement in the ucode (one 128B
    descriptor per index; stride_bytes_256 stays integral at 256B).
    num_idxs_reg must equal this core's true element count (the position of
    the trailing -1 run) so the decode-side ring reservation matches what
    the Q7 generator pushes after value-trimming.
    """
    g._assert_queue_num(queue_num)
    _in_ap = g.lower_ap_dma(in_ap, for_custom_bir_dma=True)
    _idxs_ap = g.lower_ap(idxs_ap)
    _out_ap = g.lower_ap(out_ap)
    return g.add_instruction(
        mybir.InstDMAGatherAnt(
            name=g.bass.get_next_instruction_name(),
            ins=[*_in_ap, _idxs_ap, g.lower_val_access(g.to_reg(num_idxs_reg))],
            outs=[_out_ap],
            transpose=False,
            num_idxs=num_idxs,
            elem_size=D,
            stride_bytes_256=1,
            gen_mode=0,
            single_packet=SINGLE_PACKET,
            queue_num=queue_num,
            sbuf_tokens_per_rank=0,
            sbuf_free_dim_per_rank=0,
            sbuf_free_dim_pad_per_rank=0,
            sbuf_byte_offset=0,
        )
    )


def _wrap_idxs(seg):
    """dma_gather index layout: stream pos k -> [k % 16, k // 16], x8 tiled."""
    a = seg.reshape(-1, 16).T.astype(np.int16)
    return np.tile(a, (8, 1))


def _batch_windows(b):
    return list(range(_BSTART[b], _BSTART[b] + BATCH_SIZES[b]))


def _build_plan(edge_index):
    """Host-side routing: shared static structure + per-core padded streams."""
    row = np.asarray(edge_index[0], dtype=np.int64)
    col = np.asarray(edge_index[1], dtype=np.int64)

    deg = np.bincount(row, minlength=N_NODES).astype(np.float32)
    inv_global = (1.0 / np.maximum(deg, 1.0)).astype(np.float32)

    core_of = row // NPC
    per_core = []
    cnts = np.zeros((N_CORES, NW, NCH), np.int64)
    for c in range(N_CORES):
        sel = core_of == c
        rl = (row[sel] - c * NPC).astype(np.int64)
        cl = col[sel]
        w = rl >> 7
        j = cl // CH
        np.add.at(cnts[c], (w, j), 1)
        per_core.append((rl, cl, w, j))

    # per-core exact slot offsets within each (b, j) segment; segment tile
    # count shared = max over cores (per-core tails are -1 and ucode-trimmed)
    r0c = np.zeros((N_CORES, NW, NCH), np.int64)
    n_seg = np.zeros((N_CORES, NBATCH, NCH), np.int64)
    seg_len = {}
    seg_tiles = {}
    seg_t0 = {}
    t_acc = 0
    for b in range(NBATCH):
        ws = _batch_windows(b)
        for j in range(NCH):
            for c in range(N_CORES):
                o = 0
                for w in ws:
                    r0c[c, w, j] = o
                    o += int(cnts[c, w, j])
                n_seg[c, b, j] = o
            S = max(1, int(-(-n_seg[:, b, j].max() // P)))
            seg_len[(b, j)] = S * P
            seg_tiles[(b, j)] = S
            seg_t0[(b, j)] = t_acc
            t_acc += S
    T_total = t_acc

    # union views: per (w, j), tiles touched by ANY core's run
    views = []          # (b, j, w, seg_tile)
    view_id = {}
    seg_v0 = {}
    seg_nv = {}
    for b in range(NBATCH):
        ws = _batch_windows(b)
        for j in range(NCH):
            v0 = len(views)
            seg_v0[(b, j)] = v0
            for w in ws:
                lt_lo, lt_hi = None, None
                for c in range(N_CORES):
                    cnt = int(cnts[c, w, j])
                    if cnt == 0:
                        continue
                    a = int(r0c[c, w, j])
                    lo = a >> 7
                    hi = (a + cnt - 1) >> 7
                    lt_lo = lo if lt_lo is None else min(lt_lo, lo)
                    lt_hi = hi if lt_hi is None else max(lt_hi, hi)
                if lt_lo is None:
                    continue
                for lt in range(lt_lo, lt_hi + 1):
                    view_id[(w, j, lt)] = len(views)
                    views.append((b, j, w, lt))
            seg_nv[(b, j)] = len(views) - v0
    V_total = len(views)

    # per-window matmul schedule: (j, seg_tile, seg_view)
    sched = {w: [] for w in range(NW)}
    for vi, (b, j, w, lt) in enumerate(views):
        sched[w].append((j, lt, vi - seg_v0[(b, j)]))

    # per-core streams
    cores = []
    for c in range(N_CORES):
        rl, cl, w, j = per_core[c]
        key = (w * NCH + j).astype(np.int64)
        order = np.argsort(key, kind="stable")
        key_s = key[order]
        rl_s = rl[order]
        cl_s = cl[order]
        w_s = w[order]
        j_s = j[order]
        grp_start = np.zeros(NW * NCH, np.int64)
        np.cumsum(np.bincount(key, minlength=NW * NCH), out=grp_start[0:])
        grp_start = np.concatenate([[0], grp_start[:-1]])
        rank = np.arange(len(key_s)) - grp_start[key_s]

        w2b = np.zeros(NW, np.int64)
        for bb in range(NBATCH):
            w2b[_BSTART[bb]:_BSTART[bb] + BATCH_SIZES[bb]] = bb
        b_s = w2b[w_s]
        t0_s = np.array([seg_t0[(int(bb), int(jj))]
                         for bb, jj in zip(b_s, j_s)], np.int64)
        slot_in_seg = r0c[c, w_s, j_s] + rank
        pos = t0_s * P + slot_in_seg

        stream_idx = np.zeros(T_total * P, np.int16)
        if PAD_MODE == "trim":
            # first use of each msg buffer (batches < MBUFS_MSG) keeps
            # zero-pads so every buffer byte is initialized by a gather
            for b in range(MBUFS_MSG, NBATCH):
                for jj in range(NCH):
                    base = seg_t0[(b, jj)] * P
                    stream_idx[base + int(n_seg[c, b, jj]):
                               base + seg_len[(b, jj)]] = -1
        stream_idx[pos] = (cl_s - j_s * CH).astype(np.int16)

        dv = np.full((P, V_total), -1.0, np.float32)
        lt_s = (slot_in_seg >> 7).astype(np.int64)
        vi_s = np.array([view_id[(int(ww), int(jj), int(lt))]
                         for ww, jj, lt in zip(w_s, j_s, lt_s)], np.int64)
        dv[slot_in_seg & 127, vi_s] = (rl_s & 127).astype(np.float32)
        dv = np.repeat(dv, 2, axis=1)                    # pair-duplicated

        idx_parts = []
        for b in range(NBATCH):
            for jj in range(NCH):
                base = seg_t0[(b, jj)] * P
                idx_parts.append(
                    _wrap_idxs(stream_idx[base: base + seg_len[(b, jj)]]))
        idx_wrapped = np.concatenate(idx_parts, axis=1)
        dv_bf = dv.astype(ml_dtypes.bfloat16)
        nseg_c = np.zeros((1, NBATCH * NCH), np.int32)
        for b in range(NBATCH):
            for jj in range(NCH):
                nseg_c[0, b * NCH + jj] = n_seg[c, b, jj]
        cores.append((idx_wrapped, dv_bf, nseg_c))

    shared = dict(seg_len=seg_len, seg_tiles=seg_tiles, seg_t0=seg_t0,
                  seg_v0=seg_v0, seg_nv=seg_nv, sched=sched,
                  T_total=T_total, V_total=V_total)
    return shared, cores, inv_global


def _build_program(shared, do_compute=True):
    nc = bacc.Bacc("TRN2", num_swdge_queues=4)

    seg_len = shared["seg_len"]
    seg_tiles = shared["seg_tiles"]
    seg_v0 = shared["seg_v0"]
    seg_nv = shared["seg_nv"]
    sched = shared["sched"]
    V_total = shared["V_total"]

    ST16 = sum(seg_len.values()) // 16
    Smax = max(seg_tiles.values())

    Vmax = max(seg_nv.values())

    xdup_d = nc.dram_tensor("xdup", [N_NODES, 2 * D], bf16, kind="ExternalInput")
    nseg_d = nc.dram_tensor("nseg", [1, NBATCH * NCH], mybir.dt.int32,
                            kind="ExternalInput")
    xt_d = nc.dram_tensor("xt", [D, NPC_PAD], bf16, kind="ExternalInput")
    idx_d = nc.dram_tensor("idx", [P, ST16], i16, kind="ExternalInput")
    dv_d = nc.dram_tensor("dv", [P, 2 * V_total], bf16, kind="ExternalInput")
    iota_d = nc.dram_tensor("iota", [P, Vmax * P], bf16, kind="ExternalInput")
    inv_d = nc.dram_tensor("invr", [D, NPC_PAD], bf16, kind="ExternalInput")
    wa_d = nc.dram_tensor("wa", [D, K * D], bf16, kind="ExternalInput")
    wxi_d = nc.dram_tensor("wxi", [D, K * D], bf16, kind="ExternalInput")
    out_d = nc.dram_tensor("out", [NPC_PAD, K * D], bf16, kind="ExternalOutput")

    # idx tensor column offset of each segment
    seg_idx_off = {}
    o = 0
    for b in range(NBATCH):
        for j in range(NCH):
            seg_idx_off[(b, j)] = o
            o += seg_len[(b, j)] // 16

    with tile.TileContext(nc) as tc:
        with (
            tc.tile_pool(name="const", bufs=1) as cpool,
            tc.tile_pool(name="msg", bufs=MBUFS_MSG) as mpool,
            tc.tile_pool(name="q", bufs=MBUFS_Q) as qpool,
            tc.tile_pool(name="agg", bufs=4) as apool,
            tc.tile_pool(name="outs", bufs=3) as opool,
            tc.tile_pool(name="xtb", bufs=2) as xtpool,
            tc.tile_pool(name="invb", bufs=2) as invpool,
            tc.tile_pool(name="psA", bufs=4, space="PSUM") as pApool,
            tc.tile_pool(name="psB", bufs=2, space="PSUM") as pBpool,
        ):
            if PAD_MODE == "trim":
                nseg_t = cpool.tile([1, NBATCH * NCH], mybir.dt.int32)
                nc.sync.dma_start(out=nseg_t[:], in_=nseg_d[:])
            # per-batch idx tiles so the first gather isn't gated on the
            # whole idx stream landing
            idx_bt = []
            bo = 0
            for b in range(NBATCH):
                blen = sum(seg_len[(b, j)] for j in range(NCH)) // 16
                t = cpool.tile([P, blen], i16, tag=f"idx{b}")
                nc.sync.dma_start(out=t[:], in_=idx_d[:, bo:bo + blen])
                idx_bt.append((t, bo))
                bo += blen
            dv_t = cpool.tile([P, V_total, 2], bf16)
            iota_t = cpool.tile([P, Vmax, P], bf16)
            wa_t = cpool.tile([D, K * D], bf16)
            wxi_t = cpool.tile([D, K * D], bf16)
            _dv = dv_t[:]
            nc.sync.dma_start(
                out=bass.AP(_dv.tensor, _dv.offset,
                            [_dv.ap[0], [1, 2 * V_total]]),
                in_=dv_d[:])
            _it = iota_t[:]
            nc.sync.dma_start(
                out=bass.AP(_it.tensor, _it.offset,
                            [_it.ap[0], [1, Vmax * P]]),
                in_=iota_d[:])
            nc.sync.dma_start(out=wa_t[:], in_=wa_d[:])
            nc.sync.dma_start(out=wxi_t[:], in_=wxi_d[:])

            if PAD_MODE == "trim":
                # guard only the tail tiles beyond what the first-use batch
                # (which is fully zero-padded) writes; everything else is
                # always gather-initialized before it can be read
                for j in range(NCH):
                    for k in range(MBUFS_MSG):
                        t = mpool.tile([P, Smax, D], bf16, tag=f"msg{j}")
                        s0 = seg_tiles[(k, j)]
                        if s0 < Smax:
                            nc.vector.memset(t[:, s0:, :], 0.0)

            tiles_b = {}

            def emit_gathers(b):
                msg_bj = {}
                for j in range(NCH):
                    S = seg_tiles[(b, j)]
                    L = seg_len[(b, j)]
                    idx_t, bo = idx_bt[b]
                    io = seg_idx_off[(b, j)] - bo
                    msg_t = mpool.tile([P, Smax, D], bf16, tag=f"msg{j}")
                    msg_bj[j] = msg_t
                    if PAD_MODE == "trim" and b >= MBUFS_MSG:
                        nreg = nc.gpsimd.alloc_register(f"nseg_{b}_{j}")
                        nc.gpsimd.reg_load(
                            nreg, nseg_t[0:1, b * NCH + j:b * NCH + j + 1])
                    else:
                        nreg = L
                    _dma_gather_128(
                        nc.gpsimd,
                        msg_t[:, :S, :],
                        xdup_d[CH * j:, 0:D] if j else xdup_d[:, 0:D],
                        idx_t[:, io:io + L // 16],
                        L,
                        nreg,
                        queue_num=j,
                    )
                tiles_b[b] = [msg_bj, None]

            def emit_iseq(b):
                w0 = _BSTART[b]
                nwb = BATCH_SIZES[b]
                xtb = xtpool.tile([D, WB * P], bf16, tag="xtb")
                invb = invpool.tile([D, WB * P], bf16, tag="invb")
                nc.sync.dma_start(out=xtb[:, :nwb * P],
                                  in_=xt_d[:, w0 * P:(w0 + nwb) * P])
                nc.sync.dma_start(out=invb[:, :nwb * P],
                                  in_=inv_d[:, w0 * P:(w0 + nwb) * P])
                q_bj = {}
                for j in range(NCH):
                    nv = seg_nv[(b, j)]
                    v0 = seg_v0[(b, j)]
                    q_t = qpool.tile([P, Vmax, P], bf16, tag=f"q{j}")
                    q_bj[j] = q_t
                    if nv and do_compute:
                        # pair-duplicated dv + 4-dim AP [p, v, (0, 64), (1, 2)]
                        # keeps the bf16 pair-fetch legal -> DVE 2x mode
                        dcols = dv_t[:, v0:v0 + nv, :]
                        d_b = bass.AP(dcols.tensor, dcols.offset,
                                      [dcols.ap[0], dcols.ap[1],
                                       [0, P // 2], dcols.ap[2]])
                        nc.vector.tensor_tensor(out=q_t[:, :nv, :],
                                                in0=iota_t[:, :nv, :],
                                                in1=d_b,
                                                op=mybir.AluOpType.is_equal)
                tiles_b[b][1] = (q_bj, xtb, invb)

            def emit_windows(b):
                msg_bj, (q_bj, xtb, invb) = tiles_b.pop(b)
                w0 = _BSTART[b]
                for w in _batch_windows(b):
                    if not do_compute:
                        continue
                    entries = sched[w]
                    psA = pApool.tile([D, P], f32, space="PSUM", tag="psA")
                    aggT = apool.tile([D, P], bf16, tag="agg")
                    if entries:
                        for k, (j, lt, svi) in enumerate(entries):
                            nc.tensor.matmul(
                                out=psA[:],
                                lhsT=msg_bj[j][:, lt, :],
                                rhs=q_bj[j][:, svi, :],
                                start=(k == 0),
                                stop=(k == len(entries) - 1),
                            )
                        nc.vector.tensor_tensor(
                            out=aggT[:], in0=psA[:],
                            in1=invb[:, (w - w0) * P:(w - w0 + 1) * P],
                            op=mybir.AluOpType.mult)
                    else:
                        nc.vector.memset(aggT[:], 0.0)

                    psB = pBpool.tile([P, K * D], f32, space="PSUM", tag="psB")
                    nc.tensor.matmul(out=psB[:], lhsT=aggT[:], rhs=wa_t[:],
                                     start=True, stop=False)
                    nc.tensor.matmul(out=psB[:],
                                     lhsT=xtb[:, (w - w0) * P:(w - w0 + 1) * P],
                                     rhs=wxi_t[:], start=False, stop=True)
                    out_t = opool.tile([P, K * D], bf16, tag="out")
                    nc.scalar.mul(out_t[:], psB[:], 1.0)
                    nc.sync.dma_start(out=out_d[w * P:(w + 1) * P, :],
                                      in_=out_t[:])

            # software pipeline: gathers run TWO batches ahead of the window
            # compute (covering both the SWDGE drain burst and the late DMA
            # completion semaphore), Q builds one batch ahead
            emit_gathers(0)
            emit_gathers(1)
            emit_iseq(0)
            for b in range(NBATCH):
                if b + 2 < NBATCH:
                    emit_gathers(b + 2)
                if b + 1 < NBATCH:
                    emit_iseq(b + 1)
                emit_windows(b)

    nc.compile()
    _split_sync_waits(nc)
    return nc


def kernel(x, edge_index, weights):
    x = np.asarray(x, dtype=np.float32)
    weights = np.asarray(weights, dtype=np.float32)

    shared, cores, inv_global = _build_plan(edge_index)

    shape_key = (shared["T_total"], shared["V_total"],
                 tuple(sorted(shared["seg_len"].items())),
                 tuple((w, tuple(v)) for w, v in sorted(shared["sched"].items())))
    if shape_key in _CACHE:
        nc = _CACHE[shape_key]
    else:
        nc = _build_program(shared)
        _CACHE.clear()
        _CACHE[shape_key] = nc

    x_bf = x.astype(ml_dtypes.bfloat16)
    xdup = np.concatenate([x_bf, x_bf], axis=1)          # [N, 128] bf16

    vmax = max(shared["seg_nv"].values())
    iota = np.broadcast_to(
        np.arange(P, dtype=np.float32),
        (P, vmax, P)).reshape(P, vmax * P).astype(ml_dtypes.bfloat16)

    wa = np.ascontiguousarray(
        weights[:, :D, :].transpose(1, 0, 2).reshape(D, K * D)
    ).astype(ml_dtypes.bfloat16)
    wx = weights[:, D:, :].transpose(1, 0, 2).reshape(D, K * D).copy()
    eye = np.eye(D, dtype=np.float32)
    for k in range(K):
        wx[:, k * D:(k + 1) * D] += eye
    wxi = wx.astype(ml_dtypes.bfloat16)

    in_maps = []
    for c in range(N_CORES):
        idx_wrapped, dv_bf, nseg_c = cores[c]
        xt = np.zeros((D, NPC_PAD), np.float32)
        xt[:, :NPC] = x[c * NPC:(c + 1) * NPC].T
        invr = np.zeros((D, NPC_PAD), np.float32)
        invr[:, :NPC] = inv_global[None, c * NPC:(c + 1) * NPC]
        in_maps.append({
            "xdup": xdup,
            "nseg": nseg_c,
            "xt": xt.astype(ml_dtypes.bfloat16),
            "idx": idx_wrapped,
            "dv": dv_bf,
            "iota": np.ascontiguousarray(iota),
            "invr": invr.astype(ml_dtypes.bfloat16),
            "wa": wa,
            "wxi": wxi,
        })

    res = run_bass_kernel_spmd(nc, in_maps, core_ids=list(range(N_CORES)))

    out = np.empty((N_NODES, K, D), np.float32)
    for c in range(N_CORES):
        oc = res.results[c]["out"][:NPC].astype(np.float32)  # [12500, 512]
        out[c * NPC:(c + 1) * NPC] = oc.reshape(NPC, K, D)
    return out

